# revision 18
# baseline (speedup 1.0000x reference)
"""MoE (8 experts, top-2, d=1024, N=8192) on 8 trn2 NeuronCores.

Strategy (expert-parallel with host routing/gating + slot load-balancing):
 - Host computes routing AND gates in fp64 (replicated router cost is host-side),
   pre-scales each dispatched token copy by its gate, and packs tokens into a
   tiled layout xg_t [P din-sub, T*KT*P] per core.
 - Work is balanced across cores at 128-token-tile granularity: each core holds
   TWO expert weight slots (wa: first T_A tiles, wb: remaining T-T_A tiles); a
   small host-side solver assigns (expert -> slots) so T == ceil(total_tiles/8)
   whenever feasible (SPMD instruction stream identical on all cores).
 - Device (per core): pure expert GEMM y[tok, :] = xg[tok, :] @ Wslot in bf16
   with fp32 PSUM accumulation, PSUM->SBUF copy on DVE (bf16 out), input DMAs
   on the SP queue and output DMAs on the Activation queue.
 - Host combines: out[idx] += y (+ g*b bias term), fp32.
"""

import math
import os
from contextlib import ExitStack

import ml_dtypes
import numpy as np

import concourse.bass as bass
import concourse.bacc as bacc
import concourse.mybir as mybir
import concourse.tile as tile
from concourse.bass import ts
from concourse.bass_utils import run_bass_kernel_spmd

N_EXPERTS = 8
TOP_K = 2
D = 1024
N_CORES = 8
P = 128  # partitions
KT = D // P  # number of K tiles (8)
NH = int(os.environ.get("MOE_NH", "512"))  # psum free-dim tile per matmul
G = int(os.environ.get("MOE_G", "3"))  # token tiles per group

# matmul operand dtype: "bf16" (default), "f32r", "f32"
MM_DTYPE = os.environ.get("MOE_MM_DTYPE", "bf16")
# j-outer matmul order: consecutive matmuls share the stationary x chunk
JORDER = os.environ.get("MOE_JORDER", "0") == "1"
# device y output dtype: "bf16" (default) or "f32"
OUT_DTYPE = os.environ.get("MOE_OUT_DTYPE", "bf16")

LAST_RESULTS = None  # stash of BassKernelResults for test harness inspection

_BUILD_CACHE = {}
_PLAN = {"T_A": None}  # set by _prep; _build(C, repeat) reads T_A for test.py compat


def _np_dt(name):
    return {
        "bf16": ml_dtypes.bfloat16,
        "f32r": np.float32,
        "f32": np.float32,
    }[name]


def _mybir_dt(name):
    return {
        "bf16": mybir.dt.bfloat16,
        "f32r": mybir.dt.float32r,
        "f32": mybir.dt.float32,
    }[name]


def _build(C: int, repeat: int = 1, T_A: int | None = None):
    """Build the SPMD Bass module for per-core padded token count C."""
    if T_A is None:
        T_A = _PLAN["T_A"]
        assert T_A is not None, "_prep must run before _build"
    T = C // P
    key = (C, T_A, MM_DTYPE, OUT_DTYPE, repeat, G, NH, JORDER)
    if key in _BUILD_CACHE:
        return _BUILD_CACHE[key]

    f32 = mybir.dt.float32
    mm_dt = _mybir_dt(MM_DTYPE)
    out_dt = _mybir_dt(OUT_DTYPE)

    nc = bacc.Bacc(None, target_bir_lowering=False)
    # inputs (xg_t: tiled tokens [din-sub 128, (T, KT, 128 tok)], gate pre-applied)
    xg_t = nc.declare_dram_parameter("xg_t", [P, T * KT * P], mm_dt, isOutput=False)
    wa = nc.declare_dram_parameter("wa", [D, D], mm_dt, isOutput=False)
    wb = nc.declare_dram_parameter("wb", [D, D], mm_dt, isOutput=False)
    # output (y tiled [tok-sub 128, (T, 1024 feat)])
    y = nc.declare_dram_parameter("y", [P, T * D], out_dt, isOutput=True)

    DEPTH = 4  # x prefetch depth (tiles)
    PRE_WB = 8  # x tiles prefetched before the wb load is queued

    with tile.TileContext(nc) as tc, ExitStack() as ctx:
        consts = ctx.enter_context(tc.tile_pool(name="consts", bufs=1))
        xpool = ctx.enter_context(tc.tile_pool(name="x", bufs=PRE_WB + 1))
        ypool = ctx.enter_context(tc.tile_pool(name="y", bufs=3))
        ypsum = ctx.enter_context(
            tc.tile_pool(name="ypsum", bufs=4, space=bass.MemorySpace.PSUM)
        )

        wa_sb = consts.tile([P, KT, D], mm_dt)
        wb_sb = consts.tile([P, KT, D], mm_dt)

        def load_x(t):
            xt = xpool.tile([P, KT * P], mm_dt, tag="xt")
            nc.sync.dma_start(xt[:], xg_t[:, t * KT * P : (t + 1) * KT * P])
            return xt

        def load_w(wsb, w):
            # one DMA per 128-row k-chunk so the first matmuls only wait on
            # chunk 0 (weights stream in behind the first x tile)
            for j in range(KT):
                nc.sync.dma_start(wsb[:, j, :], w[ts(j, P), :])

        def finish_tile(t, yp):
            """PSUM -> SBUF (bf16) -> DRAM. Last tile drains in NH halves to
            overlap the final matmuls; other tiles copy whole."""
            ysb = ypool.tile([P, D], out_dt, tag="ysb")
            nc.vector.tensor_copy(ysb[:], yp[:])
            # output DMA on the Activation HWDGE queue (inputs go via SP)
            nc.scalar.dma_start(y[:, t * D : (t + 1) * D], ysb[:])

        def expert_tile(t, xt, tail=False):
            wsb = wa_sb if t < T_A else wb_sb
            if tail:
                # separate PSUM tiles per NH half: the nh=0 half drains
                # (copy+DMA) while nh=1 matmuls still run, shrinking the
                # end-of-kernel tail without a WAR stall on one shared tile
                ysb = ypool.tile([P, D], out_dt, tag="ysb", name="ysb_tail")
                for nh in range(D // NH):
                    yph = ypsum.tile([P, NH], f32, tag="yp", name=f"yp_h{nh}")
                    for j in range(KT):
                        nc.tensor.matmul(
                            yph[:],
                            xt[:, j * P : (j + 1) * P],
                            wsb[:, j, ts(nh, NH)],
                            start=(j == 0),
                            stop=(j == KT - 1),
                        )
                    nc.vector.tensor_copy(ysb[:, ts(nh, NH)], yph[:])
                    nc.scalar.dma_start(
                        y[:, t * D + nh * NH : t * D + (nh + 1) * NH],
                        ysb[:, ts(nh, NH)],
                    )
                return
            yp = ypsum.tile([P, D], f32, tag="yp")
            order = (
                [(nh, j) for j in range(KT) for nh in range(D // NH)]
                if JORDER
                else [(nh, j) for nh in range(D // NH) for j in range(KT)]
            )
            for nh, j in order:
                nc.tensor.matmul(
                    yp[:, ts(nh, NH)],
                    xt[:, j * P : (j + 1) * P],
                    wsb[:, j, ts(nh, NH)],
                    start=(j == 0),
                    stop=(j == KT - 1),
                )
            finish_tile(t, yp)

        def startup_tiles(ts_list, xts):
            """j-outer across the first tiles: PE consumption of each weight
            chunk paces its DMA arrival, hiding the weight-load latency."""
            yps = {
                t: ypsum.tile([P, D], f32, tag="yp", name=f"yp_s{t}")
                for t in ts_list
            }
            for j in range(KT):
                for t in ts_list:
                    for nh in range(D // NH):
                        nc.tensor.matmul(
                            yps[t][:, ts(nh, NH)],
                            xts[t][:, j * P : (j + 1) * P],
                            wa_sb[:, j, ts(nh, NH)],
                            start=(j == 0),
                            stop=(j == KT - 1),
                        )
            for t in ts_list:
                finish_tile(t, yps[t])

        # software pipeline: first x tiles, then weights, then deep x prefetch
        rep_cm = None
        S = 0
        xts = {}

        def load_upto(n):
            n = min(n, T)
            while len(xts) == 0 or max(xts) + 1 < n:
                t = max(xts) + 1 if xts else 0
                xts[t] = load_x(t)

        if repeat > 1:
            # weights loaded once, outside the repeat loop
            load_w(wa_sb, wa)
            load_w(wb_sb, wb)
            rep_cm = tc.For_i(0, repeat, 1)
            rep_cm.__enter__()
            load_upto(DEPTH)
        else:
            S = 2 if (T >= 4 and T_A >= 2) else 0
            load_upto(max(S, 1))
            load_w(wa_sb, wa)
            # prefetch deep enough that these loads don't queue behind wb
            load_upto(PRE_WB)
            load_w(wb_sb, wb)
            if S:
                startup_tiles(list(range(S)), xts)
                for t in range(S):
                    xts.pop(t)
        for t in range(S, T):
            load_upto(t + 1 + DEPTH)
            expert_tile(t, xts.pop(t), tail=(t == T - 1))

        if rep_cm is not None:
            rep_cm.__exit__(None, None, None)

    nc.compile()
    _BUILD_CACHE[key] = nc
    return nc


def _route(x, Wr, br):
    """Host routing in fp64: per-token top-2 expert ids + softmax gates."""
    n_tokens = x.shape[0]
    logits = x.astype(np.float64) @ Wr.astype(np.float64) + br.astype(np.float64)
    i1 = np.argmax(logits, axis=1)
    l2 = logits.copy()
    l2[np.arange(n_tokens), i1] = -np.inf
    i2 = np.argmax(l2, axis=1)
    v1 = logits[np.arange(n_tokens), i1]
    v2 = l2[np.arange(n_tokens), i2]
    # softmax over the top-2 logits
    e2 = np.exp(v2 - v1)
    g1 = 1.0 / (1.0 + e2)
    g2 = e2 / (1.0 + e2)
    return i1, i2, g1, g2


def _solve_slots(tile_counts):
    """Pick (T, T_A) and per-expert slot counts so all 8 cores run T tiles
    with two weight slots (first T_A tiles -> slot A, rest -> slot B).

    Returns (T, T_A, choices) with choices[e] = (nA_e, nB_e) slot counts.
    """
    total = sum(tile_counts)
    t_lo = max(1, math.ceil(total / N_CORES))
    for T in range(t_lo, max(tile_counts) + t_lo + 1):
        for T_A in range((T + 1) // 2, T + 1):
            T_B = T - T_A
            # DP over experts on (A slots used, B slots used)
            reach = {(0, 0): []}
            for n in tile_counts:
                nreach = {}
                for (au, bu), hist in reach.items():
                    for a in range(0, N_CORES - au + 1):
                        cov = a * T_A
                        if cov >= n:
                            b = 0
                        elif T_B > 0:
                            b = math.ceil((n - cov) / T_B)
                        else:
                            continue
                        if bu + b > N_CORES:
                            continue
                        key2 = (au + a, bu + b)
                        if key2 not in nreach:
                            nreach[key2] = hist + [(a, b)]
                reach = nreach
                if not reach:
                    break
            if reach:
                key2 = min(reach.keys(), key=lambda k: k[0] + k[1])
                return T, T_A, reach[key2]
    raise RuntimeError("slot solver failed")  # unreachable: large T is feasible


def _plan_dispatch(idx_per_e):
    """Build the per-core slot plan from per-expert token index lists."""
    tile_counts = [max(1, math.ceil(len(ix) / P)) if len(ix) else 0 for ix in idx_per_e]
    T, T_A, choices = _solve_slots(tile_counts)
    T_B = T - T_A
    slots_a, slots_b = [], []
    for e, (a, b) in enumerate(choices):
        n = len(idx_per_e[e])
        pos = 0
        for kind, cap in [("A", T_A * P)] * a + [("B", T_B * P)] * b:
            take = min(n - pos, cap)
            (slots_a if kind == "A" else slots_b).append((e, pos, take))
            pos += take
        assert pos == n, f"expert {e}: assigned {pos} of {n}"
    while len(slots_a) < N_CORES:
        slots_a.append((-1, 0, 0))
    while len(slots_b) < N_CORES:
        slots_b.append((-1, 0, 0))
    cores = list(zip(slots_a, slots_b, strict=True))
    return {"T": T, "T_A": T_A, "cores": cores}


def _make_in_maps(x, W, plan, idx_per_e, gate_per_e):
    np_mm = _np_dt(MM_DTYPE)
    T, T_A = plan["T"], plan["T_A"]
    C = T * P
    in_maps = []
    for core in range(N_CORES):
        (ea, pa, na), (eb, pb, nb) = plan["cores"][core]
        xg = np.zeros((C, D), dtype=np.float32)
        if na:
            ids = idx_per_e[ea][pa : pa + na]
            xg[:na] = x[ids] * gate_per_e[ea][pa : pa + na][:, None]
        if nb:
            ids = idx_per_e[eb][pb : pb + nb]
            xg[T_A * P : T_A * P + nb] = (
                x[ids] * gate_per_e[eb][pb : pb + nb][:, None]
            )
        # partition-major layout: xg_t[p, t, j, c] = xg[t*128 + c, j*128 + p]
        xg_t = np.ascontiguousarray(
            xg.reshape(T, P, KT, P).transpose(3, 0, 2, 1).reshape(P, T * KT * P)
        ).astype(np_mm)
        in_maps.append(
            {
                "xg_t": xg_t,
                "wa": np.ascontiguousarray(W[ea if ea >= 0 else 0]).astype(np_mm),
                "wb": np.ascontiguousarray(W[eb if eb >= 0 else 0]).astype(np_mm),
            }
        )
    return in_maps


def _prep(inputs):
    x = np.asarray(inputs["x"], dtype=np.float32)
    Wr = np.asarray(inputs["Wr"], dtype=np.float32)
    br = np.asarray(inputs["br"], dtype=np.float32)
    W = np.asarray(inputs["W"], dtype=np.float32)
    b = np.asarray(inputs["b"], dtype=np.float32)
    i1, i2, g1, g2 = _route(x, Wr, br)
    idx_per_e = []
    gate_per_e = []
    for e in range(N_EXPERTS):
        m1 = i1 == e
        m2 = i2 == e
        idx = np.where(m1 | m2)[0]
        g = np.where(m1[idx], g1[idx], g2[idx]).astype(np.float32)
        idx_per_e.append(idx)
        gate_per_e.append(g)
    plan = _plan_dispatch(idx_per_e)
    plan["idx_per_e"] = idx_per_e
    plan["gate_per_e"] = gate_per_e
    _PLAN["T_A"] = plan["T_A"]
    C = plan["T"] * P
    in_maps = _make_in_maps(x, W, plan, idx_per_e, gate_per_e)
    return in_maps, plan, C, x.shape[0], b


def kernel(**inputs) -> np.ndarray:
    global LAST_RESULTS
    in_maps, plan, C, n_tokens, b = _prep(inputs)
    T, T_A = plan["T"], plan["T_A"]
    nc = _build(C, T_A=T_A)
    res = run_bass_kernel_spmd(nc, in_maps, core_ids=list(range(N_CORES)))
    LAST_RESULTS = res

    idx_per_e = plan["idx_per_e"]
    gate_per_e = plan["gate_per_e"]
    out = np.zeros((n_tokens, D), dtype=np.float32)
    for core in range(N_CORES):
        # y [P, T*D]: y[p, t*D + f] = token (t*128+p), feature f
        ye = (
            res.results[core]["y"]
            .astype(np.float32)
            .reshape(P, T, D)
            .transpose(1, 0, 2)
            .reshape(C, D)
        )
        (ea, pa, na), (eb, pb, nb) = plan["cores"][core]
        if na:
            ids = idx_per_e[ea][pa : pa + na]
            gg = gate_per_e[ea][pa : pa + na]
            out[ids] += ye[:na] + gg[:, None] * b[ea][None, :]
        if nb:
            ids = idx_per_e[eb][pb : pb + nb]
            gg = gate_per_e[eb][pb : pb + nb]
            out[ids] += ye[T_A * P : T_A * P + nb] + gg[:, None] * b[eb][None, :]
    return out
